# revision 9
# baseline (speedup 1.0000x reference)
"""DiskLoss Trainium2 kernel.

Computes the reference loss:
  pred = gather(output, ind)            # [K,33] per batch
  gt_m = even-odd rasterization of the 16-gon from target   (per object)
  dk_m = union of 15 disks (radius ceil(|pred[:,32]|)) from pred
  per_obj = 1 - inter/(union+1e-6);  loss = sum(m*per_obj)/(sum(m)+1e-6)

Sharding: data-parallel over batch B=8 -> one batch element per NeuronCore.
Each core rasterizes its own 128 objects (object-per-partition layout) and
reduces to (sum m*per_obj, sum m); host adds the 8 partial pairs.

Device algorithm (object k on SBUF partition k, coords un-offset by 32,
disk geometry additionally scaled by 1/16 so fp16 stays well-conditioned):
  - indirect-DMA gather of pred rows from output[b] transposed to [HW, C]
  - disks:  sqx[k,x,d]=((x-cx)/16)^2 (Act engine), sqy likewise;
            per 8-row chunk: slab = sqx+sqy via stride-0 broadcast
            tensor_tensor (fp16, 2x mode), min via in-place pair tree,
            dk = (min d2 <= (r/16)^2) with fused row-count accumulation
  - polygon: xint'/straddle per (y,v) in fp32; bits = (x < xint') via fp16
            tensor_tensor; parity via in-place logical_xor pair tree
  - IoU + masked reduction via PE ones-matmul over partitions
"""

import sys

if "/opt/trn_rl_repo" not in sys.path:
    sys.path.insert(0, "/opt/trn_rl_repo")

import numpy as np

B, C, H, W = 8, 33, 128, 128
K = 128
V = 16          # polygon vertices
D = 15          # disk centers
YC = 8          # disk y-chunk rows
NCH = H // YC   # 16 chunks
DS = 1.0 / 16.0  # disk coordinate scale

_CACHE = {}


def _build_nc():
    import concourse.bacc as bacc
    import concourse.mybir as mybir
    import concourse.tile as tile
    import concourse.bass as bass

    F32 = mybir.dt.float32
    F16 = mybir.dt.float16
    I32 = mybir.dt.int32
    Alu = mybir.AluOpType
    Act = mybir.ActivationFunctionType
    AX = mybir.AxisListType

    nc = bacc.Bacc("TRN2", target_bir_lowering=False, debug=False)

    # ---- DRAM I/O (per core) ----
    featT_d = nc.dram_tensor("featT", [H * W, C], F32, kind="ExternalInput")
    ind_d = nc.dram_tensor("ind", [K], I32, kind="ExternalInput")
    tgt_d = nc.dram_tensor("target", [K, C], F32, kind="ExternalInput")
    mask_d = nc.dram_tensor("mask", [K], I32, kind="ExternalInput")
    out_d = nc.dram_tensor("out", [2], F32, kind="ExternalOutput")

    # ---- SBUF ----
    pred = nc.alloc_sbuf_tensor("pred", [K, C], F32)
    tgt = nc.alloc_sbuf_tensor("tgt", [K, C], F32)
    indc = nc.alloc_sbuf_tensor("indc", [K, 1], I32)
    maski = nc.alloc_sbuf_tensor("maski", [K, 1], I32)
    maskf = nc.alloc_sbuf_tensor("maskf", [K, 1], F32)

    pxi = nc.alloc_sbuf_tensor("pxi", [128, W], I32)
    pxd = nc.alloc_sbuf_tensor("pxd", [128, W], F32)     # x'' = x-32 in [-32,96)

    negc = nc.alloc_sbuf_tensor("negc", [K, 2 * V], F32)  # [-cx_d/16 | -cy_d/16]
    sqx = nc.alloc_sbuf_tensor("sqx", [K, W, D], F16)     # (x,d) d-innermost
    sqy = nc.alloc_sbuf_tensor("sqy", [K, H, D], F16)     # (y,d)
    rsc = nc.alloc_sbuf_tensor("rsc", [K, 4], F32)
    ri = nc.alloc_sbuf_tensor("ri", [K, 1], I32)
    r2c = nc.alloc_sbuf_tensor("r2c", [K, 1], F32)

    slab = nc.alloc_sbuf_tensor("slab", [K, YC, W, D], F16)
    accq = nc.alloc_sbuf_tensor("accq", [K, YC, W], F16)
    dk = nc.alloc_sbuf_tensor("dk", [K, YC, W], F16)
    iscr = nc.alloc_sbuf_tensor("iscr", [K, YC, 64], F16)

    # polygon
    x2b = nc.alloc_sbuf_tensor("x2b", [K, V], F32)
    y2b = nc.alloc_sbuf_tensor("y2b", [K, V], F32)
    pv1 = nc.alloc_sbuf_tensor("pv1", [K, V], F32)
    pv2 = nc.alloc_sbuf_tensor("pv2", [K, V], F32)
    pv3 = nc.alloc_sbuf_tensor("pv3", [K, V], F32)
    sv = nc.alloc_sbuf_tensor("sv", [K, 64, V], F32)      # (y,v) v-innermost
    svb = nc.alloc_sbuf_tensor("svb", [K, 64, V], F32)
    xint = nc.alloc_sbuf_tensor("xint", [K, 64, V], F32)
    xint16 = nc.alloc_sbuf_tensor("xint16", [K, 64, V], F16)
    pxv16 = nc.alloc_sbuf_tensor("pxv16", [K, 64, V], F16)
    bits = nc.alloc_sbuf_tensor("bits", [K, 8, 64, V], F16)
    gt01 = nc.alloc_sbuf_tensor("gt01", [K, 64, 64], F16)
    gscr = nc.alloc_sbuf_tensor("gscr", [K, 64, 64], F16)

    # reduction buffers
    dkcols = nc.alloc_sbuf_tensor("dkcols", [K, NCH], F32)
    icols = nc.alloc_sbuf_tensor("icols", [K, 8], F32)
    stats = nc.alloc_sbuf_tensor("stats", [K, 8], F32)
    onesv = nc.alloc_sbuf_tensor("onesv", [K, 1], F32)
    colq = nc.alloc_sbuf_tensor("colq", [K, 2], F32)
    outsb = nc.alloc_sbuf_tensor("outsb", [1, 2], F32)
    psum = nc.alloc_psum_tensor("psum", [1, 2], F32)

    with tile.TileContext(nc) as tc:
        vec = nc.vector
        act = nc.scalar

        def ts(out, in0, s1, op0, s2=None, op1=None, accum=None):
            kw = {}
            if accum is not None:
                kw["accum_out"] = accum
            if op1 is not None:
                return vec.tensor_scalar(out=out, in0=in0, scalar1=s1, scalar2=s2,
                                         op0=op0, op1=op1, **kw)
            return vec.tensor_scalar(out=out, in0=in0, scalar1=s1, scalar2=None,
                                     op0=op0, **kw)

        def tt(out, in0, in1, op):
            return vec.tensor_tensor(out=out, in0=in0, in1=in1, op=op)

        # ---- P0: input DMAs + gather ----
        nc.sync.dma_start(indc.ap(), ind_d.ap().unsqueeze(1))
        nc.sync.dma_start(tgt.ap(), tgt_d.ap())
        nc.sync.dma_start(maski.ap(), mask_d.ap().unsqueeze(1))
        nc.gpsimd.indirect_dma_start(
            out=pred.ap(), out_offset=None, in_=featT_d.ap(),
            in_offset=bass.IndirectOffsetOnAxis(ap=indc.ap(), axis=0))

        # ---- P1: iotas ----
        nc.gpsimd.iota(pxi.ap(), pattern=[[1, W]], base=0, channel_multiplier=0)
        ts(pxd.ap(), pxi.ap(), 32.0, Alu.subtract)          # also int->f32
        ts(maskf.ap(), maski.ap(), 0.0, Alu.add)

        # ---- P2: r2c = (ceil(|pred[:,32]|)/16)^2  (cast-based floor) ----
        u = rsc.ap()[:, 0:1]; t = rsc.ap()[:, 1:2]; g = rsc.ap()[:, 2:3]
        ts(t, pred.ap()[:, 32:33], -1.0, Alu.mult)
        tt(u, pred.ap()[:, 32:33], t, Alu.max)              # |p|
        vec.tensor_copy(out=ri.ap(), in_=u)                 # int cast
        vec.tensor_copy(out=t, in_=ri.ap())                 # back to f32
        tt(g, t, u, Alu.is_gt)
        tt(t, t, g, Alu.subtract)                           # floor(u)
        tt(g, u, t, Alu.is_gt)
        tt(t, t, g, Alu.add)                                # ceil(u)
        ts(t, t, DS, Alu.mult)
        tt(r2c.ap(), t, t, Alu.mult)                        # (r/16)^2

        # ---- P3: per-disk squares (scaled by 1/16) ----
        ts(negc.ap()[:, 0:D], pred.ap()[:, 0:2 * D:2], -DS, Alu.mult)
        ts(negc.ap()[:, V:V + D], pred.ap()[:, 1:2 * D:2], -DS, Alu.mult)
        for d in range(D):
            act.activation(out=sqx.ap()[:, :, d], in_=pxd.ap(), func=Act.Square,
                           bias=negc.ap()[:, d:d + 1], scale=DS)
            act.activation(out=sqy.ap()[:, :, d], in_=pxd.ap(), func=Act.Square,
                           bias=negc.ap()[:, V + d:V + d + 1], scale=DS)

        # ---- P4: polygon precompute (fp32, [K, 64y, V] layout) ----
        x1v = tgt.ap()[:, 0:2 * V:2]     # [K,16]
        y1v = tgt.ap()[:, 1:2 * V:2]
        vec.tensor_copy(out=x2b.ap()[:, 0:V - 1], in_=tgt.ap()[:, 2:2 * V:2])
        vec.tensor_copy(out=x2b.ap()[:, V - 1:V], in_=tgt.ap()[:, 0:1])
        vec.tensor_copy(out=y2b.ap()[:, 0:V - 1], in_=tgt.ap()[:, 3:2 * V:2])
        vec.tensor_copy(out=y2b.ap()[:, V - 1:V], in_=tgt.ap()[:, 1:2])
        d0 = pv1.ap(); eqz = pv2.ap(); sl = pv3.ap()
        tt(d0, y2b.ap(), y1v, Alu.subtract)
        ts(eqz, d0, 0.0, Alu.is_equal)
        tt(d0, d0, eqz, Alu.add)                             # denom
        vec.reciprocal(out=eqz, in_=d0)                      # 1/denom
        tt(sl, x2b.ap(), x1v, Alu.subtract)
        tt(sl, sl, eqz, Alu.mult)                            # slope

        pyp = pxd.ap()[:, 32:96]          # y'' values 0..63
        pyp_b = pyp.unsqueeze(2).to_broadcast([K, 64, V])
        y1b_ = y1v.unsqueeze(1).to_broadcast([K, 64, V])
        y2b_ = y2b.ap().unsqueeze(1).to_broadcast([K, 64, V])
        # straddle = (y1 > y) != (y2 > y)
        tt(sv.ap(), y1b_, pyp_b, Alu.is_gt)
        tt(svb.ap(), y2b_, pyp_b, Alu.is_gt)
        tt(sv.ap(), sv.ap(), svb.ap(), Alu.not_equal)
        # xint = x1 + (y - y1)*slope ; xint' = straddle * xint (in (0,64) when straddle)
        tt(xint.ap(), pyp_b, y1b_, Alu.subtract)
        tt(xint.ap(), xint.ap(), sl.unsqueeze(1).to_broadcast([K, 64, V]), Alu.mult)
        tt(xint.ap(), xint.ap(), x1v.unsqueeze(1).to_broadcast([K, 64, V]), Alu.add)
        tt(xint16.ap(), xint.ap(), sv.ap(), Alu.mult)
        # pxv16[k, x, v] = x'' (0..63)
        ts(pxv16.ap(), pxd.ap()[:, 32:96].unsqueeze(2).to_broadcast([K, 64, V]),
           0.0, Alu.add)

        # ---- P5: polygon bits + xor-tree parity ----
        pxv_b = pxv16.ap().unsqueeze(1).to_broadcast([K, 8, 64, V])
        for sc in range(8):
            xv = xint16.ap()[:, 8 * sc:8 * sc + 8, :].unsqueeze(2) \
                .to_broadcast([K, 8, 64, V])
            tt(bits.ap(), pxv_b, xv, Alu.is_lt)              # x < xint'
            tt(bits.ap()[:, :, :, 0:8], bits.ap()[:, :, :, 0:8],
               bits.ap()[:, :, :, 8:16], Alu.logical_xor)
            tt(bits.ap()[:, :, :, 0:4], bits.ap()[:, :, :, 0:4],
               bits.ap()[:, :, :, 4:8], Alu.logical_xor)
            tt(bits.ap()[:, :, :, 0:2], bits.ap()[:, :, :, 0:2],
               bits.ap()[:, :, :, 2:4], Alu.logical_xor)
            tt(gt01.ap()[:, 8 * sc:8 * sc + 8, :],
               bits.ap()[:, :, :, 0], bits.ap()[:, :, :, 1], Alu.logical_xor)
        # area_gt (bits are exact 0/1 in fp16)
        vec.tensor_scalar(out=gscr.ap(), in0=gt01.ap(), scalar1=0.0, scalar2=None,
                          op0=Alu.add, op1=Alu.add, accum_out=stats.ap()[:, 2:3])

        # ---- P6: disks ----
        sqx_b = sqx.ap().unsqueeze(1).to_broadcast([K, YC, W, D])
        for c in range(NCH):
            sqy_b = sqy.ap()[:, YC * c:YC * (c + 1), :].unsqueeze(2) \
                .to_broadcast([K, YC, W, D])
            tt(slab.ap(), sqx_b, sqy_b, Alu.add)
            # min over 15 slots: pair 0:7 with 8:15 (slot 7 rides along), then 8->1
            tt(slab.ap()[:, :, :, 0:7], slab.ap()[:, :, :, 0:7],
               slab.ap()[:, :, :, 8:15], Alu.min)
            tt(slab.ap()[:, :, :, 0:4], slab.ap()[:, :, :, 0:4],
               slab.ap()[:, :, :, 4:8], Alu.min)
            tt(slab.ap()[:, :, :, 0:2], slab.ap()[:, :, :, 0:2],
               slab.ap()[:, :, :, 2:4], Alu.min)
            tt(accq.ap(), slab.ap()[:, :, :, 0], slab.ap()[:, :, :, 1], Alu.min)
            vec.tensor_scalar(out=dk.ap(), in0=accq.ap(), scalar1=r2c.ap(),
                              scalar2=None, op0=Alu.is_le, op1=Alu.add,
                              accum_out=dkcols.ap()[:, c:c + 1])
            yp0 = YC * c - 32
            if 0 <= yp0 and yp0 + YC <= 64:
                tt(iscr.ap(), dk.ap()[:, :, 32:96],
                   gt01.ap()[:, yp0:yp0 + YC, :], Alu.logical_and)
                vec.tensor_scalar(out=iscr.ap(), in0=iscr.ap(), scalar1=0.0,
                                  scalar2=None, op0=Alu.add, op1=Alu.add,
                                  accum_out=icols.ap()[:, c - 4:c - 3])

        # ---- P7: epilogue ----
        adk = stats.ap()[:, 0:1]; itr = stats.ap()[:, 1:2]; agt = stats.ap()[:, 2:3]
        uni = stats.ap()[:, 3:4]; den = stats.ap()[:, 4:5]; pob = stats.ap()[:, 5:6]
        vec.tensor_reduce(out=adk, in_=dkcols.ap(), axis=AX.X, op=Alu.add)
        vec.tensor_reduce(out=itr, in_=icols.ap(), axis=AX.X, op=Alu.add)
        tt(uni, adk, agt, Alu.add)
        tt(uni, uni, itr, Alu.subtract)
        ts(den, uni, 1e-6, Alu.add)
        vec.reciprocal(out=den, in_=den)
        tt(pob, itr, den, Alu.mult)
        ts(pob, pob, -1.0, Alu.mult, 1.0, Alu.add)        # 1 - inter/union
        tt(colq.ap()[:, 0:1], pob, maskf.ap(), Alu.mult)
        vec.tensor_copy(out=colq.ap()[:, 1:2], in_=maskf.ap())
        vec.memset(onesv.ap(), 1.0)
        nc.tensor.matmul(out=psum.ap(), lhsT=onesv.ap(), rhs=colq.ap(),
                         start=True, stop=True)
        vec.tensor_copy(out=outsb.ap(), in_=psum.ap())
        nc.sync.dma_start(out_d.ap().unsqueeze(0), outsb.ap())

    nc.compile()
    return nc


def _get_nc():
    if "nc" not in _CACHE:
        _CACHE["nc"] = _build_nc()
    return _CACHE["nc"]


def kernel(output, mask, ind, target, freq_mask=None):
    nc = _get_nc()
    from concourse.bass_utils import run_bass_kernel_spmd

    output = np.asarray(output, dtype=np.float32)
    target = np.asarray(target, dtype=np.float32)
    in_maps = []
    for b in range(B):
        in_maps.append({
            "featT": np.ascontiguousarray(output[b].reshape(C, H * W).T),
            "ind": np.asarray(ind[b], dtype=np.int32),
            "target": np.ascontiguousarray(target[b]),
            "mask": np.asarray(mask[b], dtype=np.int32),
        })
    res = run_bass_kernel_spmd(nc, in_maps, core_ids=list(range(B)))
    parts = np.stack([np.asarray(r["out"], dtype=np.float64) for r in res.results])
    loss = parts[:, 0].sum() / (parts[:, 1].sum() + 1e-6)
    return np.float32(loss), np.float32(0.0)


# revision 12
# speedup vs baseline: 1.0180x; 1.0180x over previous
"""DiskLoss Trainium2 kernel.

Computes the reference loss:
  pred = gather(output, ind)            # [K,33] per batch
  gt_m = even-odd rasterization of the 16-gon from target   (per object)
  dk_m = union of 15 disks (radius ceil(|pred[:,32]|)) from pred
  per_obj = 1 - inter/(union+1e-6);  loss = sum(m*per_obj)/(sum(m)+1e-6)

Sharding: data-parallel over batch B=8 -> one batch element per NeuronCore.
Each core rasterizes its own 128 objects (object-per-partition layout) and
reduces to (sum m*per_obj, sum m); host adds the 8 partial pairs.

Device algorithm (object k on SBUF partition k, coords un-offset by 32,
disk geometry additionally scaled by 1/16 so fp16 stays well-conditioned):
  - indirect-DMA gather of pred rows from output[b] transposed to [HW, C]
  - disks:  sqx[k,x,d]=((x-cx)/16)^2 (Act engine), sqy likewise;
            per 16-row chunk: slab = sqx+sqy via stride-0 broadcast
            tensor_tensor (fp16, 2x mode), min via in-place pair tree,
            dk = (min d2 <= (r/16)^2) with fused row-count accumulation
  - polygon: xint'/straddle per (y,v) in fp32; bits = (x < xint') via fp16
            tensor_tensor; parity via in-place logical_xor pair tree
  - IoU + masked reduction via PE ones-matmul over partitions
"""

import sys

if "/opt/trn_rl_repo" not in sys.path:
    sys.path.insert(0, "/opt/trn_rl_repo")

import numpy as np

B, C, H, W = 8, 33, 128, 128
K = 128
V = 16          # polygon vertices
D = 15          # disk centers
YC = 16         # disk y-chunk rows
NCH = H // YC   # 16 chunks
DS = 1.0 / 16.0  # disk coordinate scale

_CACHE = {}


def _build_nc():
    import concourse.bacc as bacc
    import concourse.mybir as mybir
    import concourse.tile as tile
    import concourse.bass as bass

    F32 = mybir.dt.float32
    F16 = mybir.dt.float16
    I32 = mybir.dt.int32
    Alu = mybir.AluOpType
    Act = mybir.ActivationFunctionType
    AX = mybir.AxisListType

    nc = bacc.Bacc("TRN2", target_bir_lowering=False, debug=False)

    # ---- DRAM I/O (per core) ----
    featT_d = nc.dram_tensor("featT", [H * W, C], F32, kind="ExternalInput")
    ind_d = nc.dram_tensor("ind", [K], I32, kind="ExternalInput")
    tgt_d = nc.dram_tensor("target", [K, C], F32, kind="ExternalInput")
    mask_d = nc.dram_tensor("mask", [K], I32, kind="ExternalInput")
    out_d = nc.dram_tensor("out", [2], F32, kind="ExternalOutput")

    # ---- SBUF ----
    pred = nc.alloc_sbuf_tensor("pred", [K, C], F32)
    tgt = nc.alloc_sbuf_tensor("tgt", [K, C], F32)
    indc = nc.alloc_sbuf_tensor("indc", [K, 1], I32)
    maski = nc.alloc_sbuf_tensor("maski", [K, 1], I32)
    maskf = nc.alloc_sbuf_tensor("maskf", [K, 1], F32)

    pxi = nc.alloc_sbuf_tensor("pxi", [128, W], I32)
    pxd = nc.alloc_sbuf_tensor("pxd", [128, W], F32)     # x'' = x-32 in [-32,96)

    negc = nc.alloc_sbuf_tensor("negc", [K, 2 * V], F32)  # [-cx_d/16 | -cy_d/16]
    sqx = nc.alloc_sbuf_tensor("sqx", [K, W, D], F16)     # (x,d) d-innermost
    sqy = nc.alloc_sbuf_tensor("sqy", [K, H, D], F16)     # (y,d)
    rsc = nc.alloc_sbuf_tensor("rsc", [K, 4], F32)
    ri = nc.alloc_sbuf_tensor("ri", [K, 1], I32)
    r2c = nc.alloc_sbuf_tensor("r2c", [K, 1], F32)

    slab = nc.alloc_sbuf_tensor("slab", [K, YC, W, D], F16)
    accq = nc.alloc_sbuf_tensor("accq", [K, YC, W], F16)
    dk = nc.alloc_sbuf_tensor("dk", [K, YC, W], F16)
    iscr = nc.alloc_sbuf_tensor("iscr", [K, YC, 64], F16)

    # polygon
    x2b = nc.alloc_sbuf_tensor("x2b", [K, V], F32)
    y2b = nc.alloc_sbuf_tensor("y2b", [K, V], F32)
    pv1 = nc.alloc_sbuf_tensor("pv1", [K, V], F32)
    pv2 = nc.alloc_sbuf_tensor("pv2", [K, V], F32)
    pv3 = nc.alloc_sbuf_tensor("pv3", [K, V], F32)
    sv = nc.alloc_sbuf_tensor("sv", [K, 64, V], F32)      # (y,v) v-innermost
    svb = nc.alloc_sbuf_tensor("svb", [K, 64, V], F32)
    xint = nc.alloc_sbuf_tensor("xint", [K, 64, V], F32)
    xint16 = nc.alloc_sbuf_tensor("xint16", [K, 64, V], F16)
    pxv16 = nc.alloc_sbuf_tensor("pxv16", [K, 64, V], F16)
    bits = nc.alloc_sbuf_tensor("bits", [K, 16, 64, V], F16)
    gt01 = nc.alloc_sbuf_tensor("gt01", [K, 64, 64], F16)
    gscr = nc.alloc_sbuf_tensor("gscr", [K, 64, 64], F16)

    # reduction buffers
    dkcols = nc.alloc_sbuf_tensor("dkcols", [K, NCH], F32)
    icols = nc.alloc_sbuf_tensor("icols", [K, max(64 // YC, 2)], F32)
    stats = nc.alloc_sbuf_tensor("stats", [K, 8], F32)
    onesv = nc.alloc_sbuf_tensor("onesv", [K, 1], F32)
    colq = nc.alloc_sbuf_tensor("colq", [K, 2], F32)
    outsb = nc.alloc_sbuf_tensor("outsb", [1, 2], F32)
    psum = nc.alloc_psum_tensor("psum", [1, 2], F32)

    with tile.TileContext(nc) as tc:
        vec = nc.vector
        act = nc.scalar

        def ts(out, in0, s1, op0, s2=None, op1=None, accum=None):
            kw = {}
            if accum is not None:
                kw["accum_out"] = accum
            if op1 is not None:
                return vec.tensor_scalar(out=out, in0=in0, scalar1=s1, scalar2=s2,
                                         op0=op0, op1=op1, **kw)
            return vec.tensor_scalar(out=out, in0=in0, scalar1=s1, scalar2=None,
                                     op0=op0, **kw)

        def tt(out, in0, in1, op):
            return vec.tensor_tensor(out=out, in0=in0, in1=in1, op=op)

        # ---- P0: input DMAs + gather ----
        nc.sync.dma_start(indc.ap(), ind_d.ap().unsqueeze(1))
        nc.sync.dma_start(tgt.ap(), tgt_d.ap())
        nc.sync.dma_start(maski.ap(), mask_d.ap().unsqueeze(1))
        nc.gpsimd.indirect_dma_start(
            out=pred.ap(), out_offset=None, in_=featT_d.ap(),
            in_offset=bass.IndirectOffsetOnAxis(ap=indc.ap(), axis=0))

        # ---- P1: iotas ----
        nc.gpsimd.iota(pxi.ap(), pattern=[[1, W]], base=0, channel_multiplier=0)
        ts(pxd.ap(), pxi.ap(), 32.0, Alu.subtract)          # also int->f32
        ts(maskf.ap(), maski.ap(), 0.0, Alu.add)

        # ---- P2: r2c = (ceil(|pred[:,32]|)/16)^2  (cast-based floor) ----
        u = rsc.ap()[:, 0:1]; t = rsc.ap()[:, 1:2]; g = rsc.ap()[:, 2:3]
        ts(t, pred.ap()[:, 32:33], -1.0, Alu.mult)
        tt(u, pred.ap()[:, 32:33], t, Alu.max)              # |p|
        vec.tensor_copy(out=ri.ap(), in_=u)                 # int cast
        vec.tensor_copy(out=t, in_=ri.ap())                 # back to f32
        tt(g, t, u, Alu.is_gt)
        tt(t, t, g, Alu.subtract)                           # floor(u)
        tt(g, u, t, Alu.is_gt)
        tt(t, t, g, Alu.add)                                # ceil(u)
        ts(t, t, DS, Alu.mult)
        tt(r2c.ap(), t, t, Alu.mult)                        # (r/16)^2

        # ---- P3: per-disk squares (scaled by 1/16) ----
        ts(negc.ap()[:, 0:D], pred.ap()[:, 0:2 * D:2], -DS, Alu.mult)
        ts(negc.ap()[:, V:V + D], pred.ap()[:, 1:2 * D:2], -DS, Alu.mult)
        for d in range(D):
            act.activation(out=sqx.ap()[:, :, d], in_=pxd.ap(), func=Act.Square,
                           bias=negc.ap()[:, d:d + 1], scale=DS)
            act.activation(out=sqy.ap()[:, :, d], in_=pxd.ap(), func=Act.Square,
                           bias=negc.ap()[:, V + d:V + d + 1], scale=DS)

        # ---- P4: polygon precompute (fp32, [K, 64y, V] layout) ----
        x1v = tgt.ap()[:, 0:2 * V:2]     # [K,16]
        y1v = tgt.ap()[:, 1:2 * V:2]
        vec.tensor_copy(out=x2b.ap()[:, 0:V - 1], in_=tgt.ap()[:, 2:2 * V:2])
        vec.tensor_copy(out=x2b.ap()[:, V - 1:V], in_=tgt.ap()[:, 0:1])
        vec.tensor_copy(out=y2b.ap()[:, 0:V - 1], in_=tgt.ap()[:, 3:2 * V:2])
        vec.tensor_copy(out=y2b.ap()[:, V - 1:V], in_=tgt.ap()[:, 1:2])
        d0 = pv1.ap(); eqz = pv2.ap(); sl = pv3.ap()
        tt(d0, y2b.ap(), y1v, Alu.subtract)
        ts(eqz, d0, 0.0, Alu.is_equal)
        tt(d0, d0, eqz, Alu.add)                             # denom
        vec.reciprocal(out=eqz, in_=d0)                      # 1/denom
        tt(sl, x2b.ap(), x1v, Alu.subtract)
        tt(sl, sl, eqz, Alu.mult)                            # slope

        pyp = pxd.ap()[:, 32:96]          # y'' values 0..63
        pyp_b = pyp.unsqueeze(2).to_broadcast([K, 64, V])
        y1b_ = y1v.unsqueeze(1).to_broadcast([K, 64, V])
        y2b_ = y2b.ap().unsqueeze(1).to_broadcast([K, 64, V])
        # straddle = (y1 > y) != (y2 > y)
        tt(sv.ap(), y1b_, pyp_b, Alu.is_gt)
        tt(svb.ap(), y2b_, pyp_b, Alu.is_gt)
        tt(sv.ap(), sv.ap(), svb.ap(), Alu.not_equal)
        # xint = x1 + (y - y1)*slope ; xint' = straddle * xint (in (0,64) when straddle)
        tt(xint.ap(), pyp_b, y1b_, Alu.subtract)
        tt(xint.ap(), xint.ap(), sl.unsqueeze(1).to_broadcast([K, 64, V]), Alu.mult)
        tt(xint.ap(), xint.ap(), x1v.unsqueeze(1).to_broadcast([K, 64, V]), Alu.add)
        tt(xint16.ap(), xint.ap(), sv.ap(), Alu.mult)
        # pxv16[k, x, v] = x'' (0..63)
        ts(pxv16.ap(), pxd.ap()[:, 32:96].unsqueeze(2).to_broadcast([K, 64, V]),
           0.0, Alu.add)

        # ---- P5: polygon bits + xor-tree parity ----
        pxv_b = pxv16.ap().unsqueeze(1).to_broadcast([K, 16, 64, V])
        for sc in range(4):
            xv = xint16.ap()[:, 16 * sc:16 * sc + 16, :].unsqueeze(2) \
                .to_broadcast([K, 16, 64, V])
            tt(bits.ap(), pxv_b, xv, Alu.is_lt)              # x < xint'
            tt(bits.ap()[:, :, :, 0:8], bits.ap()[:, :, :, 0:8],
               bits.ap()[:, :, :, 8:16], Alu.logical_xor)
            tt(bits.ap()[:, :, :, 0:4], bits.ap()[:, :, :, 0:4],
               bits.ap()[:, :, :, 4:8], Alu.logical_xor)
            tt(bits.ap()[:, :, :, 0:2], bits.ap()[:, :, :, 0:2],
               bits.ap()[:, :, :, 2:4], Alu.logical_xor)
            tt(gt01.ap()[:, 16 * sc:16 * sc + 16, :],
               bits.ap()[:, :, :, 0], bits.ap()[:, :, :, 1], Alu.logical_xor)
        # area_gt (bits are exact 0/1 in fp16)
        vec.tensor_scalar(out=gscr.ap(), in0=gt01.ap(), scalar1=0.0, scalar2=None,
                          op0=Alu.add, op1=Alu.add, accum_out=stats.ap()[:, 2:3])

        # ---- P6: disks ----
        sqx_b = sqx.ap().unsqueeze(1).to_broadcast([K, YC, W, D])
        for c in range(NCH):
            sqy_b = sqy.ap()[:, YC * c:YC * (c + 1), :].unsqueeze(2) \
                .to_broadcast([K, YC, W, D])
            tt(slab.ap(), sqx_b, sqy_b, Alu.add)
            # min over 15 slots: pair 0:7 with 8:15 (slot 7 rides along), then 8->1
            tt(slab.ap()[:, :, :, 0:7], slab.ap()[:, :, :, 0:7],
               slab.ap()[:, :, :, 8:15], Alu.min)
            tt(slab.ap()[:, :, :, 0:4], slab.ap()[:, :, :, 0:4],
               slab.ap()[:, :, :, 4:8], Alu.min)
            tt(slab.ap()[:, :, :, 0:2], slab.ap()[:, :, :, 0:2],
               slab.ap()[:, :, :, 2:4], Alu.min)
            tt(accq.ap(), slab.ap()[:, :, :, 0], slab.ap()[:, :, :, 1], Alu.min)
            vec.tensor_scalar(out=dk.ap(), in0=accq.ap(), scalar1=r2c.ap(),
                              scalar2=None, op0=Alu.is_le, op1=Alu.add,
                              accum_out=dkcols.ap()[:, c:c + 1])
            yp0 = YC * c - 32
            if 0 <= yp0 and yp0 + YC <= 64:
                tt(iscr.ap(), dk.ap()[:, :, 32:96],
                   gt01.ap()[:, yp0:yp0 + YC, :], Alu.logical_and)
                vec.tensor_scalar(out=iscr.ap(), in0=iscr.ap(), scalar1=0.0,
                                  scalar2=None, op0=Alu.add, op1=Alu.add,
                                  accum_out=icols.ap()[:, c - 32 // YC:c - 32 // YC + 1])

        # ---- P7: epilogue ----
        adk = stats.ap()[:, 0:1]; itr = stats.ap()[:, 1:2]; agt = stats.ap()[:, 2:3]
        uni = stats.ap()[:, 3:4]; den = stats.ap()[:, 4:5]; pob = stats.ap()[:, 5:6]
        vec.tensor_reduce(out=adk, in_=dkcols.ap(), axis=AX.X, op=Alu.add)
        vec.tensor_reduce(out=itr, in_=icols.ap(), axis=AX.X, op=Alu.add)
        tt(uni, adk, agt, Alu.add)
        tt(uni, uni, itr, Alu.subtract)
        ts(den, uni, 1e-6, Alu.add)
        vec.reciprocal(out=den, in_=den)
        tt(pob, itr, den, Alu.mult)
        ts(pob, pob, -1.0, Alu.mult, 1.0, Alu.add)        # 1 - inter/union
        tt(colq.ap()[:, 0:1], pob, maskf.ap(), Alu.mult)
        vec.tensor_copy(out=colq.ap()[:, 1:2], in_=maskf.ap())
        vec.memset(onesv.ap(), 1.0)
        nc.tensor.matmul(out=psum.ap(), lhsT=onesv.ap(), rhs=colq.ap(),
                         start=True, stop=True)
        vec.tensor_copy(out=outsb.ap(), in_=psum.ap())
        nc.sync.dma_start(out_d.ap().unsqueeze(0), outsb.ap())

    nc.compile()
    return nc


def _get_nc():
    if "nc" not in _CACHE:
        _CACHE["nc"] = _build_nc()
    return _CACHE["nc"]


def kernel(output, mask, ind, target, freq_mask=None):
    nc = _get_nc()
    from concourse.bass_utils import run_bass_kernel_spmd

    output = np.asarray(output, dtype=np.float32)
    target = np.asarray(target, dtype=np.float32)
    in_maps = []
    for b in range(B):
        in_maps.append({
            "featT": np.ascontiguousarray(output[b].reshape(C, H * W).T),
            "ind": np.asarray(ind[b], dtype=np.int32),
            "target": np.ascontiguousarray(target[b]),
            "mask": np.asarray(mask[b], dtype=np.int32),
        })
    res = run_bass_kernel_spmd(nc, in_maps, core_ids=list(range(B)))
    parts = np.stack([np.asarray(r["out"], dtype=np.float64) for r in res.results])
    loss = parts[:, 0].sum() / (parts[:, 1].sum() + 1e-6)
    return np.float32(loss), np.float32(0.0)


# revision 15
# speedup vs baseline: 1.0295x; 1.0113x over previous
"""DiskLoss Trainium2 kernel.

Computes the reference loss:
  pred = gather(output, ind)            # [K,33] per batch
  gt_m = even-odd rasterization of the 16-gon from target   (per object)
  dk_m = union of 15 disks (radius ceil(|pred[:,32]|)) from pred
  per_obj = 1 - inter/(union+1e-6);  loss = sum(m*per_obj)/(sum(m)+1e-6)

Sharding: data-parallel over batch B=8 -> one batch element per NeuronCore.
Each core rasterizes its own 128 objects (object-per-partition layout) and
reduces to (sum m*per_obj, sum m); host adds the 8 partial pairs.

Device algorithm (object k on SBUF partition k, coords un-offset by 32,
disk geometry additionally scaled by 1/16 so fp16 stays well-conditioned):
  - indirect-DMA gather of pred rows from output[b] transposed to [HW, C]
  - disks:  sqx[k,x,d]=((x-cx)/16)^2 (Act engine), sqy likewise;
            per 16-row chunk: slab = sqx+sqy via stride-0 broadcast
            tensor_tensor (fp16, 2x mode), min via in-place pair tree,
            dk = (min d2 <= (r/16)^2) with fused row-count accumulation
  - polygon: xint'/straddle per (y,v) in fp32; bits = (x < xint') via fp16
            tensor_tensor; parity via in-place logical_xor pair tree
  - IoU + masked reduction via PE ones-matmul over partitions
"""

import sys

if "/opt/trn_rl_repo" not in sys.path:
    sys.path.insert(0, "/opt/trn_rl_repo")

import numpy as np

B, C, H, W = 8, 33, 128, 128
K = 128
V = 16          # polygon vertices
D = 15          # disk centers
YC = 16         # disk y-chunk rows
NCH = H // YC   # 16 chunks
DS = 1.0 / 16.0  # disk coordinate scale

_CACHE = {}


def _build_nc():
    import concourse.bacc as bacc
    import concourse.mybir as mybir
    import concourse.tile as tile
    import concourse.bass as bass

    F32 = mybir.dt.float32
    F16 = mybir.dt.float16
    I32 = mybir.dt.int32
    Alu = mybir.AluOpType
    Act = mybir.ActivationFunctionType
    AX = mybir.AxisListType

    nc = bacc.Bacc("TRN2", target_bir_lowering=False, debug=False)

    # ---- DRAM I/O (per core) ----
    featT_d = nc.dram_tensor("featT", [H * W, C], F32, kind="ExternalInput")
    ind_d = nc.dram_tensor("ind", [K], I32, kind="ExternalInput")
    tgt_d = nc.dram_tensor("target", [K, C], F32, kind="ExternalInput")
    mask_d = nc.dram_tensor("mask", [K], I32, kind="ExternalInput")
    out_d = nc.dram_tensor("out", [2], F32, kind="ExternalOutput")

    # ---- SBUF ----
    pred = nc.alloc_sbuf_tensor("pred", [K, C], F32)
    tgt = nc.alloc_sbuf_tensor("tgt", [K, C], F32)
    indc = nc.alloc_sbuf_tensor("indc", [K, 1], I32)
    maski = nc.alloc_sbuf_tensor("maski", [K, 1], I32)
    maskf = nc.alloc_sbuf_tensor("maskf", [K, 1], F32)

    pxi = nc.alloc_sbuf_tensor("pxi", [128, W], I32)
    pxd = nc.alloc_sbuf_tensor("pxd", [128, W], F32)     # x'' = x-32 in [-32,96)

    negc = nc.alloc_sbuf_tensor("negc", [K, 2 * V], F32)  # [-cx_d/16 | -cy_d/16]
    sqx = nc.alloc_sbuf_tensor("sqx", [K, W, D], F16)     # (x,d) d-innermost
    sqy = nc.alloc_sbuf_tensor("sqy", [K, H, D], F16)     # (y,d)
    rsc = nc.alloc_sbuf_tensor("rsc", [K, 4], F32)
    ri = nc.alloc_sbuf_tensor("ri", [K, 1], I32)
    r2c = nc.alloc_sbuf_tensor("r2c", [K, 1], F32)

    slab = nc.alloc_sbuf_tensor("slab", [K, YC, W, D], F16)
    accq = nc.alloc_sbuf_tensor("accq", [K, YC, W], F16)
    dk = nc.alloc_sbuf_tensor("dk", [K, YC, W], F16)
    iscr = nc.alloc_sbuf_tensor("iscr", [K, YC, 64], F16)

    # polygon
    x2b = nc.alloc_sbuf_tensor("x2b", [K, V], F32)
    y2b = nc.alloc_sbuf_tensor("y2b", [K, V], F32)
    pv1 = nc.alloc_sbuf_tensor("pv1", [K, V], F32)
    pv2 = nc.alloc_sbuf_tensor("pv2", [K, V], F32)
    pv3 = nc.alloc_sbuf_tensor("pv3", [K, V], F32)
    sv = nc.alloc_sbuf_tensor("sv", [K, 64, V], F32)      # (y,v) v-innermost
    svb = nc.alloc_sbuf_tensor("svb", [K, 64, V], F32)
    xint = nc.alloc_sbuf_tensor("xint", [K, 64, V], F32)
    xint16 = nc.alloc_sbuf_tensor("xint16", [K, 64, V], F16)
    pxv16 = nc.alloc_sbuf_tensor("pxv16", [K, 64, V], F16)
    bits = nc.alloc_sbuf_tensor("bits", [K, 16, 64, V], F16)
    gt01 = nc.alloc_sbuf_tensor("gt01", [K, 64, 64], F16)
    gscr = nc.alloc_sbuf_tensor("gscr", [K, 64, 64], F16)

    # reduction buffers
    dkcols = nc.alloc_sbuf_tensor("dkcols", [K, NCH], F32)
    icols = nc.alloc_sbuf_tensor("icols", [K, max(64 // YC, 2)], F32)
    stats = nc.alloc_sbuf_tensor("stats", [K, 8], F32)
    onesv = nc.alloc_sbuf_tensor("onesv", [K, 1], F32)
    colq = nc.alloc_sbuf_tensor("colq", [K, 2], F32)
    outsb = nc.alloc_sbuf_tensor("outsb", [1, 2], F32)
    psum = nc.alloc_psum_tensor("psum", [1, 2], F32)

    with tile.TileContext(nc) as tc:
        vec = nc.vector
        act = nc.scalar

        def ts(out, in0, s1, op0, s2=None, op1=None, accum=None):
            kw = {}
            if accum is not None:
                kw["accum_out"] = accum
            if op1 is not None:
                return vec.tensor_scalar(out=out, in0=in0, scalar1=s1, scalar2=s2,
                                         op0=op0, op1=op1, **kw)
            return vec.tensor_scalar(out=out, in0=in0, scalar1=s1, scalar2=None,
                                     op0=op0, **kw)

        def tt(out, in0, in1, op):
            return vec.tensor_tensor(out=out, in0=in0, in1=in1, op=op)

        # ---- P0: input DMAs + gather ----
        nc.sync.dma_start(indc.ap(), ind_d.ap().unsqueeze(1))
        nc.sync.dma_start(tgt.ap(), tgt_d.ap())
        nc.sync.dma_start(maski.ap(), mask_d.ap().unsqueeze(1))
        nc.gpsimd.indirect_dma_start(
            out=pred.ap(), out_offset=None, in_=featT_d.ap(),
            in_offset=bass.IndirectOffsetOnAxis(ap=indc.ap(), axis=0))

        # ---- P1: iotas ----
        nc.gpsimd.iota(pxi.ap(), pattern=[[1, W]], base=0, channel_multiplier=0)
        ts(pxd.ap(), pxi.ap(), 32.0, Alu.subtract)          # also int->f32
        ts(maskf.ap(), maski.ap(), 0.0, Alu.add)

        # ---- P2: r2c = (ceil(|pred[:,32]|)/16)^2  (cast-based floor) ----
        u = rsc.ap()[:, 0:1]; t = rsc.ap()[:, 1:2]; g = rsc.ap()[:, 2:3]
        ts(t, pred.ap()[:, 32:33], -1.0, Alu.mult)
        tt(u, pred.ap()[:, 32:33], t, Alu.max)              # |p|
        vec.tensor_copy(out=ri.ap(), in_=u)                 # int cast
        vec.tensor_copy(out=t, in_=ri.ap())                 # back to f32
        tt(g, t, u, Alu.is_gt)
        tt(t, t, g, Alu.subtract)                           # floor(u)
        tt(g, u, t, Alu.is_gt)
        tt(t, t, g, Alu.add)                                # ceil(u)
        ts(t, t, DS, Alu.mult)
        tt(r2c.ap(), t, t, Alu.mult)                        # (r/16)^2

        # ---- P3: per-disk squares (scaled by 1/16) ----
        ts(negc.ap()[:, 0:D], pred.ap()[:, 0:2 * D:2], -DS, Alu.mult)
        ts(negc.ap()[:, V:V + D], pred.ap()[:, 1:2 * D:2], -DS, Alu.mult)
        for d in range(D):
            act.activation(out=sqx.ap()[:, :, d], in_=pxd.ap(), func=Act.Square,
                           bias=negc.ap()[:, d:d + 1], scale=DS)
            act.activation(out=sqy.ap()[:, :, d], in_=pxd.ap(), func=Act.Square,
                           bias=negc.ap()[:, V + d:V + d + 1], scale=DS)

        # ---- P4: polygon precompute (fp32, [K, 64y, V] layout) ----
        x1v = tgt.ap()[:, 0:2 * V:2]     # [K,16]
        y1v = tgt.ap()[:, 1:2 * V:2]
        vec.tensor_copy(out=x2b.ap()[:, 0:V - 1], in_=tgt.ap()[:, 2:2 * V:2])
        vec.tensor_copy(out=x2b.ap()[:, V - 1:V], in_=tgt.ap()[:, 0:1])
        vec.tensor_copy(out=y2b.ap()[:, 0:V - 1], in_=tgt.ap()[:, 3:2 * V:2])
        vec.tensor_copy(out=y2b.ap()[:, V - 1:V], in_=tgt.ap()[:, 1:2])
        d0 = pv1.ap(); eqz = pv2.ap(); sl = pv3.ap()
        tt(d0, y2b.ap(), y1v, Alu.subtract)
        ts(eqz, d0, 0.0, Alu.is_equal)
        tt(d0, d0, eqz, Alu.add)                             # denom
        vec.reciprocal(out=eqz, in_=d0)                      # 1/denom
        tt(sl, x2b.ap(), x1v, Alu.subtract)
        tt(sl, sl, eqz, Alu.mult)                            # slope

        pyp = pxd.ap()[:, 32:96]          # y'' values 0..63
        pyp_b = pyp.unsqueeze(2).to_broadcast([K, 64, V])
        y1b_ = y1v.unsqueeze(1).to_broadcast([K, 64, V])
        y2b_ = y2b.ap().unsqueeze(1).to_broadcast([K, 64, V])
        # straddle = (y1 > y) != (y2 > y)
        tt(sv.ap(), y1b_, pyp_b, Alu.is_gt)
        tt(svb.ap(), y2b_, pyp_b, Alu.is_gt)
        tt(sv.ap(), sv.ap(), svb.ap(), Alu.not_equal)
        # xint = x1 + (y - y1)*slope ; xint' = straddle * xint (in (0,64) when straddle)
        tt(xint.ap(), pyp_b, y1b_, Alu.subtract)
        tt(xint.ap(), xint.ap(), sl.unsqueeze(1).to_broadcast([K, 64, V]), Alu.mult)
        tt(xint.ap(), xint.ap(), x1v.unsqueeze(1).to_broadcast([K, 64, V]), Alu.add)
        tt(xint16.ap(), xint.ap(), sv.ap(), Alu.mult)
        # pxv16[k, x, v] = x'' (0..63)
        ts(pxv16.ap(), pxd.ap()[:, 32:96].unsqueeze(2).to_broadcast([K, 64, V]),
           0.0, Alu.add)

        # ---- P5: polygon bits + xor-tree parity ----
        pxv_b = pxv16.ap().unsqueeze(1).to_broadcast([K, 16, 64, V])
        for sc in range(4):
            xv = xint16.ap()[:, 16 * sc:16 * sc + 16, :].unsqueeze(2) \
                .to_broadcast([K, 16, 64, V])
            tt(bits.ap(), pxv_b, xv, Alu.is_lt)              # x < xint'
            tt(bits.ap()[:, :, :, 0:8], bits.ap()[:, :, :, 0:8],
               bits.ap()[:, :, :, 8:16], Alu.logical_xor)
            tt(bits.ap()[:, :, :, 0:4], bits.ap()[:, :, :, 0:4],
               bits.ap()[:, :, :, 4:8], Alu.logical_xor)
            tt(bits.ap()[:, :, :, 0:2], bits.ap()[:, :, :, 0:2],
               bits.ap()[:, :, :, 2:4], Alu.logical_xor)
            tt(gt01.ap()[:, 16 * sc:16 * sc + 16, :],
               bits.ap()[:, :, :, 0], bits.ap()[:, :, :, 1], Alu.logical_xor)
        # area_gt (bits are exact 0/1 in fp16)
        act.activation(out=gscr.ap(), in_=gt01.ap(), func=Act.Identity,
                       bias=0.0, scale=1.0, accum_out=stats.ap()[:, 2:3])

        # ---- P6: disks ----
        sqx_b = sqx.ap().unsqueeze(1).to_broadcast([K, YC, W, D])
        for c in range(NCH):
            sqy_b = sqy.ap()[:, YC * c:YC * (c + 1), :].unsqueeze(2) \
                .to_broadcast([K, YC, W, D])
            tt(slab.ap(), sqx_b, sqy_b, Alu.add)
            # min over 15 slots: pair 0:7 with 8:15 (slot 7 rides along), then 8->1
            tt(slab.ap()[:, :, :, 0:7], slab.ap()[:, :, :, 0:7],
               slab.ap()[:, :, :, 8:15], Alu.min)
            tt(slab.ap()[:, :, :, 0:4], slab.ap()[:, :, :, 0:4],
               slab.ap()[:, :, :, 4:8], Alu.min)
            tt(slab.ap()[:, :, :, 0:2], slab.ap()[:, :, :, 0:2],
               slab.ap()[:, :, :, 2:4], Alu.min)
            tt(accq.ap(), slab.ap()[:, :, :, 0], slab.ap()[:, :, :, 1], Alu.min)
            # dk sign on the Act engine: sgn = sign(r^2 - acc) (+1 inside),
            # with the fused Act accumulator giving sum(sgn) per partition.
            act.activation(out=dk.ap(), in_=accq.ap(), func=Act.Sign,
                           bias=r2c.ap(), scale=-1.0,
                           accum_out=dkcols.ap()[:, c:c + 1])
            yp0 = YC * c - 32
            if 0 <= yp0 and yp0 + YC <= 64:
                tt(iscr.ap(), dk.ap()[:, :, 32:96],
                   gt01.ap()[:, yp0:yp0 + YC, :], Alu.mult)
                vec.tensor_scalar(out=iscr.ap(), in0=iscr.ap(), scalar1=0.0,
                                  scalar2=None, op0=Alu.add, op1=Alu.add,
                                  accum_out=icols.ap()[:, c - 32 // YC:c - 32 // YC + 1])

        # ---- P7: epilogue ----
        adk = stats.ap()[:, 0:1]; itr = stats.ap()[:, 1:2]; agt = stats.ap()[:, 2:3]
        uni = stats.ap()[:, 3:4]; den = stats.ap()[:, 4:5]; pob = stats.ap()[:, 5:6]
        vec.tensor_reduce(out=adk, in_=dkcols.ap(), axis=AX.X, op=Alu.add)
        ts(adk, adk, 0.5, Alu.mult, float(H * W // 2), Alu.add)
        vec.tensor_reduce(out=itr, in_=icols.ap(), axis=AX.X, op=Alu.add)
        tt(itr, itr, agt, Alu.add)
        ts(itr, itr, 0.5, Alu.mult)
        tt(uni, adk, agt, Alu.add)
        tt(uni, uni, itr, Alu.subtract)
        ts(den, uni, 1e-6, Alu.add)
        vec.reciprocal(out=den, in_=den)
        tt(pob, itr, den, Alu.mult)
        ts(pob, pob, -1.0, Alu.mult, 1.0, Alu.add)        # 1 - inter/union
        tt(colq.ap()[:, 0:1], pob, maskf.ap(), Alu.mult)
        vec.tensor_copy(out=colq.ap()[:, 1:2], in_=maskf.ap())
        vec.memset(onesv.ap(), 1.0)
        nc.tensor.matmul(out=psum.ap(), lhsT=onesv.ap(), rhs=colq.ap(),
                         start=True, stop=True)
        vec.tensor_copy(out=outsb.ap(), in_=psum.ap())
        nc.sync.dma_start(out_d.ap().unsqueeze(0), outsb.ap())

    nc.compile()
    return nc


def _get_nc():
    if "nc" not in _CACHE:
        _CACHE["nc"] = _build_nc()
    return _CACHE["nc"]


def kernel(output, mask, ind, target, freq_mask=None):
    nc = _get_nc()
    from concourse.bass_utils import run_bass_kernel_spmd

    output = np.asarray(output, dtype=np.float32)
    target = np.asarray(target, dtype=np.float32)
    in_maps = []
    for b in range(B):
        in_maps.append({
            "featT": np.ascontiguousarray(output[b].reshape(C, H * W).T),
            "ind": np.asarray(ind[b], dtype=np.int32),
            "target": np.ascontiguousarray(target[b]),
            "mask": np.asarray(mask[b], dtype=np.int32),
        })
    res = run_bass_kernel_spmd(nc, in_maps, core_ids=list(range(B)))
    parts = np.stack([np.asarray(r["out"], dtype=np.float64) for r in res.results])
    loss = parts[:, 0].sum() / (parts[:, 1].sum() + 1e-6)
    return np.float32(loss), np.float32(0.0)


# revision 17
# speedup vs baseline: 1.0311x; 1.0016x over previous
"""DiskLoss Trainium2 kernel.

Computes the reference loss:
  pred = gather(output, ind)            # [K,33] per batch
  gt_m = even-odd rasterization of the 16-gon from target   (per object)
  dk_m = union of 15 disks (radius ceil(|pred[:,32]|)) from pred
  per_obj = 1 - inter/(union+1e-6);  loss = sum(m*per_obj)/(sum(m)+1e-6)

Sharding: data-parallel over batch B=8 -> one batch element per NeuronCore.
Each core rasterizes its own 128 objects (object-per-partition layout) and
reduces to (sum m*per_obj, sum m); host adds the 8 partial pairs.

Device algorithm (object k on SBUF partition k, coords un-offset by 32,
disk geometry additionally scaled by 1/16 so fp16 stays well-conditioned):
  - indirect-DMA gather of pred rows from output[b] transposed to [HW, C]
  - disks:  sqx[k,x,d]=((x-cx)/16)^2 (Act engine), sqy likewise;
            per 16-row chunk: slab = sqx+sqy via stride-0 broadcast
            tensor_tensor (fp16, 2x mode), min via in-place pair tree,
            dk = (min d2 <= (r/16)^2) with fused row-count accumulation
  - polygon: xint'/straddle per (y,v) in fp32; bits = (x < xint') via fp16
            tensor_tensor; parity via in-place logical_xor pair tree
  - IoU + masked reduction via PE ones-matmul over partitions
"""

import sys

if "/opt/trn_rl_repo" not in sys.path:
    sys.path.insert(0, "/opt/trn_rl_repo")

import numpy as np

B, C, H, W = 8, 33, 128, 128
K = 128
V = 16          # polygon vertices
D = 15          # disk centers
YC = 16         # disk y-chunk rows
NCH = H // YC   # 16 chunks
DS = 1.0 / 16.0  # disk coordinate scale

_CACHE = {}


def _build_nc():
    import concourse.bacc as bacc
    import concourse.mybir as mybir
    import concourse.tile as tile
    import concourse.bass as bass

    F32 = mybir.dt.float32
    F16 = mybir.dt.float16
    I32 = mybir.dt.int32
    Alu = mybir.AluOpType
    Act = mybir.ActivationFunctionType
    AX = mybir.AxisListType

    nc = bacc.Bacc("TRN2", target_bir_lowering=False, debug=False)

    # ---- DRAM I/O (per core) ----
    featT_d = nc.dram_tensor("featT", [H * W, C], F32, kind="ExternalInput")
    ind_d = nc.dram_tensor("ind", [K], I32, kind="ExternalInput")
    tgt_d = nc.dram_tensor("target", [K, C], F32, kind="ExternalInput")
    mask_d = nc.dram_tensor("mask", [K], I32, kind="ExternalInput")
    out_d = nc.dram_tensor("out", [2], F32, kind="ExternalOutput")

    # ---- SBUF ----
    pred = nc.alloc_sbuf_tensor("pred", [K, C], F32)
    tgt = nc.alloc_sbuf_tensor("tgt", [K, C], F32)
    indc = nc.alloc_sbuf_tensor("indc", [K, 1], I32)
    maski = nc.alloc_sbuf_tensor("maski", [K, 1], I32)
    maskf = nc.alloc_sbuf_tensor("maskf", [K, 1], F32)

    pxi = nc.alloc_sbuf_tensor("pxi", [128, W], I32)
    pxd = nc.alloc_sbuf_tensor("pxd", [128, W], F32)     # x'' = x-32 in [-32,96)

    negc = nc.alloc_sbuf_tensor("negc", [K, 2 * V], F32)  # [-cx_d/16 | -cy_d/16]
    sqx = nc.alloc_sbuf_tensor("sqx", [K, W, D], F16)     # (x,d) d-innermost
    sqy = nc.alloc_sbuf_tensor("sqy", [K, H, D], F16)     # (y,d)
    rsc = nc.alloc_sbuf_tensor("rsc", [K, 4], F32)
    ri = nc.alloc_sbuf_tensor("ri", [K, 1], I32)
    r2c = nc.alloc_sbuf_tensor("r2c", [K, 1], F32)

    slab = nc.alloc_sbuf_tensor("slab", [K, YC, W, D], F16)
    accq = nc.alloc_sbuf_tensor("accq", [K, YC, W], F16)
    dk4 = nc.alloc_sbuf_tensor("dk4", [K, 4, YC, W], F16)
    dko = nc.alloc_sbuf_tensor("dko", [K, YC, W], F16)
    iscr = nc.alloc_sbuf_tensor("iscr", [K, YC, 64], F16)

    # polygon
    x2b = nc.alloc_sbuf_tensor("x2b", [K, V], F32)
    y2b = nc.alloc_sbuf_tensor("y2b", [K, V], F32)
    pv1 = nc.alloc_sbuf_tensor("pv1", [K, V], F32)
    pv2 = nc.alloc_sbuf_tensor("pv2", [K, V], F32)
    pv3 = nc.alloc_sbuf_tensor("pv3", [K, V], F32)
    sv = nc.alloc_sbuf_tensor("sv", [K, 64, V], F32)      # (y,v) v-innermost
    svb = nc.alloc_sbuf_tensor("svb", [K, 64, V], F32)
    xint = nc.alloc_sbuf_tensor("xint", [K, 64, V], F32)
    xint16 = nc.alloc_sbuf_tensor("xint16", [K, 64, V], F16)
    pxv16 = nc.alloc_sbuf_tensor("pxv16", [K, 64, V], F16)
    bits = nc.alloc_sbuf_tensor("bits", [K, 16, 64, V], F16)
    gt01 = nc.alloc_sbuf_tensor("gt01", [K, 64, 64], F16)
    gscr = nc.alloc_sbuf_tensor("gscr", [K, 64, 64], F16)

    # reduction buffers
    dkcols = nc.alloc_sbuf_tensor("dkcols", [K, NCH], F32)
    icols = nc.alloc_sbuf_tensor("icols", [K, max(64 // YC, 2)], F32)
    stats = nc.alloc_sbuf_tensor("stats", [K, 8], F32)
    onesv = nc.alloc_sbuf_tensor("onesv", [K, 1], F32)
    colq = nc.alloc_sbuf_tensor("colq", [K, 2], F32)
    outsb = nc.alloc_sbuf_tensor("outsb", [1, 2], F32)
    psum = nc.alloc_psum_tensor("psum", [1, 2], F32)

    with tile.TileContext(nc) as tc:
        vec = nc.vector
        act = nc.scalar

        def ts(out, in0, s1, op0, s2=None, op1=None, accum=None):
            kw = {}
            if accum is not None:
                kw["accum_out"] = accum
            if op1 is not None:
                return vec.tensor_scalar(out=out, in0=in0, scalar1=s1, scalar2=s2,
                                         op0=op0, op1=op1, **kw)
            return vec.tensor_scalar(out=out, in0=in0, scalar1=s1, scalar2=None,
                                     op0=op0, **kw)

        def tt(out, in0, in1, op):
            return vec.tensor_tensor(out=out, in0=in0, in1=in1, op=op)

        # ---- P0: input DMAs + gather ----
        nc.sync.dma_start(indc.ap(), ind_d.ap().unsqueeze(1))
        nc.sync.dma_start(tgt.ap(), tgt_d.ap())
        nc.sync.dma_start(maski.ap(), mask_d.ap().unsqueeze(1))
        nc.gpsimd.indirect_dma_start(
            out=pred.ap(), out_offset=None, in_=featT_d.ap(),
            in_offset=bass.IndirectOffsetOnAxis(ap=indc.ap(), axis=0))

        # ---- P1: iotas ----
        nc.gpsimd.iota(pxi.ap(), pattern=[[1, W]], base=0, channel_multiplier=0)
        ts(pxd.ap(), pxi.ap(), 32.0, Alu.subtract)          # also int->f32
        ts(maskf.ap(), maski.ap(), 0.0, Alu.add)

        # ---- P3: per-disk squares (scaled by 1/16) ----
        ts(negc.ap()[:, 0:D], pred.ap()[:, 0:2 * D:2], -DS, Alu.mult)
        ts(negc.ap()[:, V:V + D], pred.ap()[:, 1:2 * D:2], -DS, Alu.mult)
        for d in range(D):
            act.activation(out=sqx.ap()[:, :, d], in_=pxd.ap(), func=Act.Square,
                           bias=negc.ap()[:, d:d + 1], scale=DS)
            act.activation(out=sqy.ap()[:, :, d], in_=pxd.ap(), func=Act.Square,
                           bias=negc.ap()[:, V + d:V + d + 1], scale=DS)

        # ---- P4: polygon precompute (fp32, [K, 64y, V] layout) ----
        x1v = tgt.ap()[:, 0:2 * V:2]     # [K,16]
        y1v = tgt.ap()[:, 1:2 * V:2]
        vec.tensor_copy(out=x2b.ap()[:, 0:V - 1], in_=tgt.ap()[:, 2:2 * V:2])
        vec.tensor_copy(out=x2b.ap()[:, V - 1:V], in_=tgt.ap()[:, 0:1])
        vec.tensor_copy(out=y2b.ap()[:, 0:V - 1], in_=tgt.ap()[:, 3:2 * V:2])
        vec.tensor_copy(out=y2b.ap()[:, V - 1:V], in_=tgt.ap()[:, 1:2])
        d0 = pv1.ap(); eqz = pv2.ap(); sl = pv3.ap()
        tt(d0, y2b.ap(), y1v, Alu.subtract)
        ts(eqz, d0, 0.0, Alu.is_equal)
        tt(d0, d0, eqz, Alu.add)                             # denom
        vec.reciprocal(out=eqz, in_=d0)                      # 1/denom
        tt(sl, x2b.ap(), x1v, Alu.subtract)
        tt(sl, sl, eqz, Alu.mult)                            # slope

        pyp = pxd.ap()[:, 32:96]          # y'' values 0..63
        pyp_b = pyp.unsqueeze(2).to_broadcast([K, 64, V])
        y1b_ = y1v.unsqueeze(1).to_broadcast([K, 64, V])
        y2b_ = y2b.ap().unsqueeze(1).to_broadcast([K, 64, V])
        # straddle = (y1 > y) != (y2 > y)
        tt(sv.ap(), y1b_, pyp_b, Alu.is_gt)
        tt(svb.ap(), y2b_, pyp_b, Alu.is_gt)
        tt(sv.ap(), sv.ap(), svb.ap(), Alu.not_equal)
        # xint = x1 + (y - y1)*slope ; xint' = straddle * xint (in (0,64) when straddle)
        tt(xint.ap(), pyp_b, y1b_, Alu.subtract)
        tt(xint.ap(), xint.ap(), sl.unsqueeze(1).to_broadcast([K, 64, V]), Alu.mult)
        tt(xint.ap(), xint.ap(), x1v.unsqueeze(1).to_broadcast([K, 64, V]), Alu.add)
        tt(xint16.ap(), xint.ap(), sv.ap(), Alu.mult)
        # pxv16[k, x, v] = x'' (0..63)
        ts(pxv16.ap(), pxd.ap()[:, 32:96].unsqueeze(2).to_broadcast([K, 64, V]),
           0.0, Alu.add)

        # ---- P5: polygon bits + xor-tree parity ----
        pxv_b = pxv16.ap().unsqueeze(1).to_broadcast([K, 16, 64, V])
        for sc in range(4):
            xv = xint16.ap()[:, 16 * sc:16 * sc + 16, :].unsqueeze(2) \
                .to_broadcast([K, 16, 64, V])
            tt(bits.ap(), pxv_b, xv, Alu.is_lt)              # x < xint'
            tt(bits.ap()[:, :, :, 0:8], bits.ap()[:, :, :, 0:8],
               bits.ap()[:, :, :, 8:16], Alu.logical_xor)
            tt(bits.ap()[:, :, :, 0:4], bits.ap()[:, :, :, 0:4],
               bits.ap()[:, :, :, 4:8], Alu.logical_xor)
            tt(bits.ap()[:, :, :, 0:2], bits.ap()[:, :, :, 0:2],
               bits.ap()[:, :, :, 2:4], Alu.logical_xor)
            tt(gt01.ap()[:, 16 * sc:16 * sc + 16, :],
               bits.ap()[:, :, :, 0], bits.ap()[:, :, :, 1], Alu.logical_xor)
        # area_gt (bits are exact 0/1 in fp16)
        act.activation(out=gscr.ap(), in_=gt01.ap(), func=Act.Identity,
                       bias=0.0, scale=1.0, accum_out=stats.ap()[:, 2:3])

        # ---- P2: r2c = (ceil(|pred[:,32]|)/16)^2  (cast-based floor) ----
        u = rsc.ap()[:, 0:1]; t = rsc.ap()[:, 1:2]; g = rsc.ap()[:, 2:3]
        ts(t, pred.ap()[:, 32:33], -1.0, Alu.mult)
        tt(u, pred.ap()[:, 32:33], t, Alu.max)              # |p|
        vec.tensor_copy(out=ri.ap(), in_=u)                 # int cast
        vec.tensor_copy(out=t, in_=ri.ap())                 # back to f32
        tt(g, t, u, Alu.is_gt)
        tt(t, t, g, Alu.subtract)                           # floor(u)
        tt(g, u, t, Alu.is_gt)
        tt(t, t, g, Alu.add)                                # ceil(u)
        ts(t, t, DS, Alu.mult)
        tt(r2c.ap(), t, t, Alu.mult)                        # (r/16)^2

        # ---- P6: disks ----
        sqx_b = sqx.ap().unsqueeze(1).to_broadcast([K, YC, W, D])
        for c in range(NCH):
            sqy_b = sqy.ap()[:, YC * c:YC * (c + 1), :].unsqueeze(2) \
                .to_broadcast([K, YC, W, D])
            tt(slab.ap(), sqx_b, sqy_b, Alu.add)
            # min over 15 slots: pair 0:7 with 8:15 (slot 7 rides along), then 8->1
            tt(slab.ap()[:, :, :, 0:7], slab.ap()[:, :, :, 0:7],
               slab.ap()[:, :, :, 8:15], Alu.min)
            tt(slab.ap()[:, :, :, 0:4], slab.ap()[:, :, :, 0:4],
               slab.ap()[:, :, :, 4:8], Alu.min)
            tt(slab.ap()[:, :, :, 0:2], slab.ap()[:, :, :, 0:2],
               slab.ap()[:, :, :, 2:4], Alu.min)
            tt(accq.ap(), slab.ap()[:, :, :, 0], slab.ap()[:, :, :, 1], Alu.min)
            # dk sign on the Act engine: sgn = sign(r^2 - acc) (+1 inside),
            # with the fused Act accumulator giving sum(sgn) per partition.
            # Polygon-region chunks keep their sign plane in dk4 so the
            # intersection pass can run after the loop, hiding Act latency.
            yp0 = YC * c - 32
            inside = 0 <= yp0 and yp0 + YC <= 64
            sgn_out = dk4.ap()[:, (c - 32 // YC) % 4, :, :] if inside else dko.ap()
            act.activation(out=sgn_out, in_=accq.ap(), func=Act.Sign,
                           bias=r2c.ap(), scale=-1.0,
                           accum_out=dkcols.ap()[:, c:c + 1])
        for j in range(64 // YC):
            tt(iscr.ap(), dk4.ap()[:, j, :, 32:96],
               gt01.ap()[:, YC * j:YC * (j + 1), :], Alu.mult)
            vec.tensor_scalar(out=iscr.ap(), in0=iscr.ap(), scalar1=0.0,
                              scalar2=None, op0=Alu.add, op1=Alu.add,
                              accum_out=icols.ap()[:, j:j + 1])

        # ---- P7: epilogue ----
        adk = stats.ap()[:, 0:1]; itr = stats.ap()[:, 1:2]; agt = stats.ap()[:, 2:3]
        uni = stats.ap()[:, 3:4]; den = stats.ap()[:, 4:5]; pob = stats.ap()[:, 5:6]
        vec.tensor_reduce(out=adk, in_=dkcols.ap(), axis=AX.X, op=Alu.add)
        ts(adk, adk, 0.5, Alu.mult, float(H * W // 2), Alu.add)
        vec.tensor_reduce(out=itr, in_=icols.ap(), axis=AX.X, op=Alu.add)
        tt(itr, itr, agt, Alu.add)
        ts(itr, itr, 0.5, Alu.mult)
        tt(uni, adk, agt, Alu.add)
        tt(uni, uni, itr, Alu.subtract)
        ts(den, uni, 1e-6, Alu.add)
        vec.reciprocal(out=den, in_=den)
        tt(pob, itr, den, Alu.mult)
        ts(pob, pob, -1.0, Alu.mult, 1.0, Alu.add)        # 1 - inter/union
        tt(colq.ap()[:, 0:1], pob, maskf.ap(), Alu.mult)
        vec.tensor_copy(out=colq.ap()[:, 1:2], in_=maskf.ap())
        vec.memset(onesv.ap(), 1.0)
        nc.tensor.matmul(out=psum.ap(), lhsT=onesv.ap(), rhs=colq.ap(),
                         start=True, stop=True)
        vec.tensor_copy(out=outsb.ap(), in_=psum.ap())
        nc.sync.dma_start(out_d.ap().unsqueeze(0), outsb.ap())

    nc.compile()
    return nc


def _get_nc():
    if "nc" not in _CACHE:
        _CACHE["nc"] = _build_nc()
    return _CACHE["nc"]


def kernel(output, mask, ind, target, freq_mask=None):
    nc = _get_nc()
    from concourse.bass_utils import run_bass_kernel_spmd

    output = np.asarray(output, dtype=np.float32)
    target = np.asarray(target, dtype=np.float32)
    in_maps = []
    for b in range(B):
        in_maps.append({
            "featT": np.ascontiguousarray(output[b].reshape(C, H * W).T),
            "ind": np.asarray(ind[b], dtype=np.int32),
            "target": np.ascontiguousarray(target[b]),
            "mask": np.asarray(mask[b], dtype=np.int32),
        })
    res = run_bass_kernel_spmd(nc, in_maps, core_ids=list(range(B)))
    parts = np.stack([np.asarray(r["out"], dtype=np.float64) for r in res.results])
    loss = parts[:, 0].sum() / (parts[:, 1].sum() + 1e-6)
    return np.float32(loss), np.float32(0.0)
